# revision 17
# baseline (speedup 1.0000x reference)
"""AxialAttention TRN2 Bass kernel — 8-core data-parallel over batch, bf16.

Reference math (per batch element b, per head h):
  qkv = x @ w_qkv;  q,k,v split; heads of dh=64
  S[m, n] = q_m . k_n / 8   (m, n over 1024 = 32x32 positions)
  attn = softmax over y only, where n = x*32 + y  (groups of 32 consecutive n)
  out[m] = sum_n attn[m, n] v[n];  y = out @ w_out + b_out

Per-core layout (core c handles batch c), all matmul operands bf16:
  xT    [512, 1024] bf16  host-pretransposed x (one merged SBUF tile)
  qkT   [1024, 1024] (q rows 0-511, k rows 512-1023), tiles [128,1024] bf16
  v     [1024, 512] natural bf16
  E^T   per (head, nt): [128 n, 1024 m] = exp(S^T/8) bf16 (ACT from PSUM),
        S^T matmuls row-packed two heads per array pass
  Z4    per head-pair: [128, 512] f32 PSUM; 4 col-packed accumulation chains,
        strip j=2*hh+mc holds the 32 group sums of head (2s+hh), m-half mc
  rz    = reciprocal(Z4) -> bf16, bounced to DRAM scratch with interleaved
        rows (2*x + hh) so zrep rebuilds with 3-dim stride-0 broadcast DMAs
  Zrep  per pair: [128, 16384] bf16 SBUF (cols = nt*2048 + hh*1024 + m)
  E'    = E * Zrep (DVE tensor_tensor, all-bf16 2x mode)
  outT  per pair: [128 (2 heads x 64 d), 1024 m]; PV col-packed 2 heads
  y     = outT.T @ w_out + b_out (bias via K=1 matmul), copies on gpsimd/DVE
"""
import numpy as np

B, H, W, DIM = 8, 32, 32, 512
HEADS, DH = 8, 64
M = H * W          # 1024 query/key positions
NT = M // 128      # 8 n-tiles / m-tiles

_CACHE = {}


def _build(loop_n=1):
    import concourse.bass as bass
    import concourse.mybir as mybir
    import concourse.tile as tile
    from concourse import bacc
    from contextlib import ExitStack

    F32 = mybir.dt.float32
    BF16 = mybir.dt.bfloat16
    EXP = mybir.ActivationFunctionType.Exp

    nc = bacc.Bacc("TRN2", target_bir_lowering=False, debug=False,
                   enable_asserts=False, num_devices=8)
    xT = nc.dram_tensor("xT", [DIM, M], BF16, kind="ExternalInput").ap()
    w_qkv = nc.dram_tensor("w_qkv", [DIM, 3 * DIM], BF16, kind="ExternalInput").ap()
    w_out = nc.dram_tensor("w_out", [DIM, DIM], BF16, kind="ExternalInput").ap()
    b_out = nc.dram_tensor("b_out", [1, DIM], BF16, kind="ExternalInput").ap()
    iz = nc.dram_tensor("iz", [128, 256], BF16, kind="ExternalInput").ap()
    ones_r = nc.dram_tensor("ones_r", [1, M], BF16, kind="ExternalInput").ap()
    y = nc.dram_tensor("y", [M, DIM], F32, kind="ExternalOutput").ap()

    with tile.TileContext(nc) as tc, ExitStack() as top:
        if loop_n > 1:
            top.enter_context(tc.For_i(0, loop_n, 1))
        persist = top.enter_context(tc.tile_pool(name="persist", bufs=1))

        # ---- persistent constants ----
        iz_sb = persist.tile([128, 256], BF16, tag="iz")
        nc.sync.dma_start(out=iz_sb, in_=iz)
        ones_sb = persist.tile([1, M], BF16, tag="ones")
        nc.sync.dma_start(out=ones_sb, in_=ones_r)
        bout_sb = persist.tile([1, DIM], BF16, tag="bout")
        nc.sync.dma_start(out=bout_sb, in_=b_out)
        wo_sb = persist.tile([128, 4 * DIM], BF16, tag="wo")
        nc.sync.dma_start(out=wo_sb,
                          in_=w_out.rearrange("(kt dp) o -> dp kt o", kt=4))

        # ---- persistent activations ----
        qkT_sb = [persist.tile([128, M], BF16, tag=f"qkT{i}", name=f"qkT{i}")
                  for i in range(8)]
        v_sb = [persist.tile([128, DIM], BF16, tag=f"v{i}", name=f"v{i}")
                for i in range(NT)]
        outT_sb = [persist.tile([128, M], BF16, tag=f"outT{i}", name=f"outT{i}")
                   for i in range(4)]

        # ---- attention pools (live through front for pipelining) ----
        e_pool = top.enter_context(tc.tile_pool(name="e_sb", bufs=2))
        zq_pool = top.enter_context(tc.tile_pool(name="zq_sb", bufs=2))
        stg_pool = top.enter_context(tc.tile_pool(name="stg_sb", bufs=2))
        scr_pool = top.enter_context(tc.tile_pool(name="scr_dram", bufs=2,
                                                  space="DRAM"))
        s_pool = top.enter_context(tc.tile_pool(name="s_ps", bufs=2, space="PSUM"))
        z_pool = top.enter_context(tc.tile_pool(name="z_ps", bufs=1, space="PSUM"))
        pv_pool = top.enter_context(tc.tile_pool(name="pv_ps", bufs=2, space="PSUM"))
        y_pool = top.enter_context(tc.tile_pool(name="y_sb", bufs=3))

        E = {}
        zq = {}

        # ================= FRONT (chunk emitters) =================
        front_ctx = ExitStack()
        fsb = front_ctx.enter_context(tc.tile_pool(name="front_sb", bufs=1))

        xT_sb = fsb.tile([128, 4 * M], BF16, tag="xTall")
        wq_sb = fsb.tile([128, 4 * 3 * DIM], BF16, tag="wqall")

        def front_loads():
            nc.sync.dma_start(
                out=xT_sb, in_=xT.rearrange("(kc dp) m -> dp kc m", kc=4))
            nc.sync.dma_start(
                out=wq_sb, in_=w_qkv.rearrange("(kt dp) f -> dp kt f", kt=4))

        def wq_ap(kt, f0, f1):
            return wq_sb[:, 1536 * kt + f0:1536 * kt + f1]

        def xt_ap(kt, m0, m1):
            return xT_sb[:, M * kt + m0:M * kt + m1]

        def mk_qkT(ft, eng):
            def emit(ft=ft, eng=eng):
                qk_ps = s_pool.tile([128, M], F32, tag="s", name=f"qkps{ft}")
                for mc in range(2):
                    for kt in range(4):
                        nc.tensor.matmul(
                            qk_ps[:, 512 * mc:512 * (mc + 1)],
                            wq_ap(kt, 128 * ft, 128 * (ft + 1)),
                            xt_ap(kt, 512 * mc, 512 * (mc + 1)),
                            start=(kt == 0), stop=(kt == 3))
                if eng == "v":
                    nc.vector.tensor_copy(qkT_sb[ft], qk_ps)
                else:
                    nc.scalar.copy(qkT_sb[ft], qk_ps)
            return emit

        def mk_v(pt):
            def emit(pt=pt):
                v_ps = pv_pool.tile([128, DIM], F32, tag="pv", name=f"vps{pt}")
                for kt in range(4):
                    nc.tensor.matmul(v_ps,
                                     xt_ap(kt, 128 * pt, 128 * (pt + 1)),
                                     wq_ap(kt, 1024, 1536),
                                     start=(kt == 0), stop=(kt == 3))
                nc.vector.tensor_copy(v_sb[pt], v_ps)
            return emit

        # ================= ATTENTION (chunk emitters) =================
        def alpha_chunks(s):
            """S^T + exp for head pair s: E[s,hh,nt] bf16 [128 n, 1024 m]."""
            out = []
            for nt in range(NT):
                for hh in range(2):
                    def emit(s=s, nt=nt, hh=hh):
                        off = 64 * hh
                        qt = qkT_sb[s]
                        kt_ = qkT_sb[4 + s]
                        s_ps = s_pool.tile([128, M], F32, tag="s",
                                           name=f"sps_{s}_{nt}_{hh}")
                        for mc in range(2):
                            nc.tensor.matmul(
                                s_ps[:, 512 * mc:512 * (mc + 1)],
                                kt_[off:off + 64, 128 * nt:128 * (nt + 1)],
                                qt[off:off + 64, 512 * mc:512 * (mc + 1)],
                                start=True, stop=True,
                                tile_position=(off, 0))
                        e = e_pool.tile([128, M], BF16, tag=f"E{hh}_{nt}",
                                        name=f"E_{s}_{nt}_{hh}")
                        nc.scalar.activation(out=e, in_=s_ps, func=EXP,
                                             scale=0.125)
                        E[s, hh, nt] = e
                    out.append(emit)
            return out

        def beta_chunks(s, tail_projs=None):
            """Z -> recip -> bounce -> Zrep broadcast -> mul -> PV for pair s."""
            chunks = []

            z_unit = {}

            def z_block(s=s, z_unit=z_unit):
                z_ps = z_pool.tile([128, 512], F32, tag="z4", name=f"z4_{s}")
                for nt in range(NT):
                    for hh in range(2):
                        for mc in range(2):
                            j = 2 * hh + mc
                            nc.tensor.matmul(
                                z_ps[32 * j:32 * (j + 1), :],
                                iz_sb[:, 32 * nt:32 * (nt + 1)],
                                E[s, hh, nt][:, 512 * mc:512 * (mc + 1)],
                                start=(nt == 0), stop=(nt == 7),
                                tile_position=(0, 32 * j),
                                skip_group_check=True)
                z_unit["z_ps"] = z_ps
            chunks.append(z_block)

            def recip_bounce(s=s, z_unit=z_unit):
                rz4f = stg_pool.tile([128, 512], F32, tag="rz4f",
                                     name=f"rz4f_{s}")
                nc.vector.reciprocal_approx_fast(out=rz4f, in_=z_unit["z_ps"])
                rzb4 = stg_pool.tile([128, 512], BF16, tag="rzb4",
                                     name=f"rzb4_{s}")
                nc.vector.tensor_copy(rzb4, rz4f)
                # DRAM scratch rows are interleaved: row = 2*x + hh
                scr = scr_pool.tile([64, M], BF16, tag="scr", name=f"scr_{s}")
                z_unit["scr"] = scr
                scr_v = scr.rearrange("(x two) (mc m) -> two mc x m",
                                      two=2, mc=2)
                for hh in range(2):
                    for mc in range(2):
                        p0 = 64 * hh + 32 * mc
                        nc.sync.dma_start(out=scr_v[hh:hh + 1, mc:mc + 1],
                                          in_=rzb4[p0:p0 + 32, :])
            chunks.append(recip_bounce)

            def mk_zq(s=s):
                zq[s] = zq_pool.tile([128, 16 * M], BF16, tag="zq",
                                     name=f"zq_{s}")
            chunks.append(mk_zq)
            for nt in range(NT):
                def bcast(s=s, nt=nt, z_unit=z_unit):
                    scr = z_unit["scr"]
                    src = (scr[8 * nt:8 * nt + 8, :]
                           .rearrange("(r two) m -> r (two m)", r=4)
                           .rearrange("r (o hm) -> r o hm", o=1)
                           .broadcast_to((4, 32, 2 * M)))
                    nc.sync.dma_start(
                        out=zq[s][:, 2 * M * nt:2 * M * (nt + 1)],
                        in_=src)
                chunks.append(bcast)

            # normalize: E' = E * Zrep  (all-bf16 SBUF, DVE 2x)
            for nt in range(NT):
                for hh in range(2):
                    def mul(s=s, nt=nt, hh=hh):
                        c0 = 2 * M * nt + M * hh
                        nc.vector.tensor_mul(
                            out=E[s, hh, nt], in0=E[s, hh, nt],
                            in1=zq[s][:, c0:c0 + M])
                    chunks.append(mul)

            # PV: col-packed pair of heads per PSUM tile
            for mc in range(2):
                unit = {}

                def pv_open(s=s, mc=mc, unit=unit):
                    unit["pv"] = pv_pool.tile([128, 512], F32, tag="pv",
                                              name=f"pv_{s}_{mc}")
                chunks.append(pv_open)
                for nt in range(NT):
                    def pv_step(s=s, mc=mc, nt=nt, unit=unit):
                        for hh in range(2):
                            h = 2 * s + hh
                            nc.tensor.matmul(
                                unit["pv"][64 * hh:64 * (hh + 1), :],
                                v_sb[nt][:, 64 * h:64 * (h + 1)],
                                E[s, hh, nt][:, 512 * mc:512 * (mc + 1)],
                                start=(nt == 0), stop=(nt == 7),
                                tile_position=(0, 64 * hh),
                                skip_group_check=True)
                    chunks.append(pv_step)

                def pv_out(s=s, mc=mc, unit=unit):
                    nc.vector.tensor_copy(
                        outT_sb[s][:, 512 * mc:512 * (mc + 1)],
                        unit["pv"])
                chunks.append(pv_out)
                if tail_projs is not None:
                    chunks.extend(tail_projs[4 * mc:4 * (mc + 1)])
            return chunks

        # ================= PROJ (chunk emitters) =================
        def mk_proj(mt, eng):
            def emit(mt=mt, eng=eng):
                p = pv_pool.tile([128, DIM], F32, tag="pv", name=f"pj{mt}")
                for kt in range(4):
                    nc.tensor.matmul(p,
                                     outT_sb[kt][:, 128 * mt:128 * (mt + 1)],
                                     wo_sb[:, 512 * kt:512 * (kt + 1)],
                                     start=(kt == 0), stop=False)
                nc.tensor.matmul(p, ones_sb[:, 128 * mt:128 * (mt + 1)],
                                 bout_sb, start=False, stop=True)
                y_sb = y_pool.tile([128, DIM], F32, tag="y", name=f"ysb{mt}")
                if eng == "v":
                    nc.vector.tensor_copy(y_sb, p)
                else:
                    nc.scalar.copy(y_sb, p)
                nc.sync.dma_start(out=y[128 * mt:128 * (mt + 1), :], in_=y_sb)
            return emit

        # ================= EMISSION SCHEDULE =================
        def interleave(a, b):
            """Emit a and b interleaved proportionally (a is the pacing list)."""
            na, nb = len(a), len(b)
            if not a:
                for f in b:
                    f()
                return
            ratio = nb / na
            bi = 0.0
            for i, f in enumerate(a):
                f()
                target = (i + 1) * ratio
                while bi < target and int(bi) < nb:
                    b[int(bi)]()
                    bi += 1
            for j in range(int(bi), nb):
                b[j]()

        front_loads()
        # qkT tiles for pair 0 first, then the rest of front interleaved with A(0)
        mk_qkT(0, "v")()
        mk_qkT(4, "g")()
        rest_front = []
        for s_ in range(1, 4):
            rest_front.append(mk_qkT(s_, "v" if s_ % 2 else "g"))
            rest_front.append(mk_qkT(4 + s_, "g" if s_ % 2 else "v"))
        for pt in range(NT):
            rest_front.append(mk_v(pt))
        interleave(alpha_chunks(0), rest_front)
        front_ctx.close()

        # steady pipeline: alpha(s+1) interleaved with beta(s)
        for s in range(1, 4):
            interleave(alpha_chunks(s), beta_chunks(s - 1))
        # last beta: interleave the first proj chunks after PV mc=0 completes
        projs = [mk_proj(mt, "v" if mt % 2 else "g") for mt in range(NT)]
        for f in beta_chunks(3, tail_projs=projs):
            f()

    nc.compile()
    return nc


def _consts():
    import ml_dtypes
    izv = np.zeros((128, 256), np.float32)
    for nt in range(8):
        for n in range(128):
            izv[n, 32 * nt + 4 * nt + n // 32] = 1.0
    ones = np.ones((1, M), np.float32)
    return (izv.astype(ml_dtypes.bfloat16), ones.astype(ml_dtypes.bfloat16))


def _in_maps(x, w_qkv, w_out, b_out):
    import ml_dtypes
    bf16 = ml_dtypes.bfloat16
    izv, ones = _consts()
    x = np.asarray(x, np.float32).astype(bf16)
    wq = np.asarray(w_qkv, np.float32).astype(bf16)
    wo = np.asarray(w_out, np.float32).astype(bf16)
    bo = np.asarray(b_out, np.float32).astype(bf16).reshape(1, DIM)
    maps = []
    for c in range(8):
        maps.append({
            "xT": np.ascontiguousarray(x[c].reshape(M, DIM).T),
            "w_qkv": wq, "w_out": wo, "b_out": bo,
            "iz": izv, "ones_r": ones,
        })
    return maps


def kernel(x, w_qkv, w_out, b_out):
    from concourse import bass_utils
    if "nc" not in _CACHE:
        _CACHE["nc"] = _build()
    nc = _CACHE["nc"]
    in_maps = _in_maps(x, w_qkv, w_out, b_out)
    res = bass_utils.run_bass_kernel_spmd(nc, in_maps, core_ids=list(range(8)))
    out = np.stack([res.results[c]["y"].reshape(H, W, DIM) for c in range(8)])
    return out
